# revision 9
# baseline (speedup 1.0000x reference)
"""Relational GNN layer  y = sum_r A_r @ X @ W_r^T  on 8 trn2 NeuronCores.

Sharding: relation-parallel. Core c handles relation c:
    Y_c = A_c @ Z_c,   Z_c = X @ W_c^T     (A_c: [N, N], Z_c: [N, F])
Host sums the 8 partial [N, F] outputs.

The kernel is at the per-core HBM roofline (~330 GB/s measured; ring
count does not change it), so the structure minimizes DMA bytes and
keeps the stream saturated end to end:
  - A ships as 1 byte/element fp8 after mean-centering A = 0.5 + B,
    at = fp8(16 * B); the 0.5-mean path is restored exactly via the
    host rank-1 term cs = 8 * W_c @ colsum(X).
  - Z_c is tiny (0.69 MiB vs 16.8 MiB for A) and is computed host-side
    and shipped directly as z16 (fp16) / z8 (e4m3) — no xt/wt transfer,
    no on-device Z phase gating the PE.
  - Column-block order: the full 32-k-tile contraction runs per 512-wide
    output block, so each PSUM bank finishes while A still streams and
    its copy-out + output DMA hide inside the stream (no serial tail).

Hybrid precision (per-matmul perf on trn2 PE, measured):
  - fp16 x e3m4 tiles: 512-col matmul = 518 cyc (1 k-tile).
  - e4m3 x e4m3 DoubleRow: same 518 cyc for 2 k-tiles (2 fp8
    weights/cell; full 2x, no observed penalty).
First NFPK=14 k-tiles of each contraction run accurate (fp16 x e3m4),
the remaining 18 run DoubleRow. Sim-measured end-to-end relative error:
1.696e-2 (gate 2e-2), deterministic (fixed PRNG seed). Per-block PE
time (6.0 us at a throttled 2.0 GHz) stays at/under the per-block DMA
time (~6-6.8 us), so the kernel stays DMA-bound even when throttled.

Shapes hardcoded for R=8, N=4096, F_IN=F_OUT=128.
"""

import numpy as np
import ml_dtypes

R, N, F = 8, 4096, 128
JBLK = 32                # contraction k-tiles of 128
NFPK = 14                # k-tiles per block in accurate fp16 x e3m4 mode
NDRK = JBLK - NFPK       # k-tiles per block in fp8 DoubleRow mode (pairs)
NCORES = 8
NQ = N // 512            # 8 psum banks / 512-wide output blocks
ASCALE = 16.0
NWARM = 6

_CACHE = {}


def _build_program():
    import concourse.mybir as mybir
    import concourse.tile as tile
    from concourse import bacc

    dt = mybir.dt
    alu = mybir.AluOpType
    nc = bacc.Bacc("TRN2", target_bir_lowering=False, debug=False)

    a3 = nc.dram_tensor("a3", [128, NQ * NFPK * 512], dt.float8e3, kind="ExternalInput").ap()
    a4 = nc.dram_tensor("a4", [128, NQ * NDRK * 512], dt.float8e4, kind="ExternalInput").ap()
    z16d = nc.dram_tensor("z16d", [128, NFPK * 128], dt.float16, kind="ExternalInput").ap()
    z8d = nc.dram_tensor("z8d", [128, NDRK * 128], dt.float8e4, kind="ExternalInput").ap()
    cs = nc.dram_tensor("cs", [F, 1], dt.float32, kind="ExternalInput").ap()
    yt = nc.dram_tensor("yt", [F, N], dt.float16, kind="ExternalOutput").ap()

    with tile.TileContext(nc) as tc:
        with (
            tc.sbuf_pool(name="const", bufs=1) as cpool,
            tc.sbuf_pool(name="a3blocks", bufs=6) as pa3,
            tc.sbuf_pool(name="a4blocks", bufs=6) as pa4,
            tc.psum_pool(name="yp", bufs=8) as yp,
        ):
            accs = [
                yp.tile([128, 512], dt.float32, tag="yacc", name=f"yacc{q}")
                for q in range(NQ)
            ]

            # Warm the PE p-state while the lead-in DMAs run; depends only
            # on a DVE memset.
            wdum = cpool.tile([128, 128], dt.float16)
            nc.vector.memset(wdum[:], 0.0)
            for _ in range(NWARM):
                nc.tensor.matmul(
                    accs[0][:, 0:128], lhsT=wdum[:], rhs=wdum[:],
                    start=True, stop=True,
                )

            # z tiles lead both rings (small; gate the whole PE stream).
            z16 = cpool.tile([128, NFPK, 128], dt.float16)
            nc.sync.dma_start(out=z16[:], in_=z16d)
            z8 = cpool.tile([128, NDRK, 128], dt.float8e4)
            nc.scalar.dma_start(out=z8[:], in_=z8d)
            colsum_s = cpool.tile([128, 1], dt.float32)
            nc.scalar.dma_start(out=colsum_s[:], in_=cs)

            # A block DMAs: per output block q, the fp16-mode part a3q
            # [128, NFPK, 512] and the DoubleRow part a4q [128, NDRK, 512],
            # each split in half across BOTH rings so arrival matches the
            # PE's in-order consumption; pool bufs throttle against the PE.
            yt_sb = cpool.tile([128, N], dt.float16)
            H3 = NFPK // 2
            H4 = NDRK // 2
            for q in range(NQ):
                a3q = pa3.tile([128, NFPK, 512], dt.float8e3, tag="a3b",
                               name=f"a3b{q}")
                a4q = pa4.tile([128, NDRK, 512], dt.float8e4, tag="a4b",
                               name=f"a4b{q}")
                e0, e1 = (nc.sync, nc.scalar) if q % 2 == 0 else (nc.scalar, nc.sync)
                o3 = q * NFPK * 512
                o4 = q * NDRK * 512
                if q == 0:
                    # split q0 with a sync bias: scalar starts ~1.5us later
                    # (engine preamble), so sync covers the early bytes.
                    nc.sync.dma_start(out=a3q[:], in_=a3[:, o3 : o3 + NFPK * 512])
                    nc.sync.dma_start(out=a4q[:, 0:H4, :],
                                      in_=a4[:, o4 : o4 + H4 * 512])
                    nc.scalar.dma_start(out=a4q[:, H4:NDRK, :],
                                        in_=a4[:, o4 + H4 * 512 : o4 + NDRK * 512])
                elif q == NQ - 1:
                    # split the last block across both rings so the final
                    # matmuls overlap the last bytes of the stream
                    e0.dma_start(out=a3q[:, 0:H3, :],
                                 in_=a3[:, o3 : o3 + H3 * 512])
                    e1.dma_start(out=a3q[:, H3:NFPK, :],
                                 in_=a3[:, o3 + H3 * 512 : o3 + NFPK * 512])
                    e1.dma_start(out=a4q[:, 0:H4, :],
                                 in_=a4[:, o4 : o4 + H4 * 512])
                    e0.dma_start(out=a4q[:, H4:NDRK, :],
                                 in_=a4[:, o4 + H4 * 512 : o4 + NDRK * 512])
                else:
                    e0.dma_start(out=a3q[:], in_=a3[:, o3 : o3 + NFPK * 512])
                    e1.dma_start(out=a4q[:], in_=a4[:, o4 : o4 + NDRK * 512])

                # Full contraction for output block q: acc[f, i] =
                # sum_j z[j, f] * at[j, i],  j = all 32 k-tiles.
                for kt in range(NFPK):
                    nc.tensor.matmul(
                        accs[q][:],
                        lhsT=z16[:, kt : kt + 1, :],
                        rhs=a3q[:, kt, :],
                        start=(kt == 0),
                        stop=False,
                    )
                for u in range(NDRK // 2):
                    nc.tensor.matmul(
                        accs[q][:],
                        lhsT=z8[:, 2 * u : 2 * u + 2, :],
                        rhs=a4q[:, 2 * u : 2 * u + 2, :],
                        start=False,
                        stop=(u == NDRK // 2 - 1),
                        perf_mode=mybir.MatmulPerfMode.DoubleRow,
                    )

                # copy-out block q (+cs mean correction, fp32 -> fp16) and
                # its output chunk, hidden under the next blocks' stream.
                nc.vector.tensor_scalar(
                    out=yt_sb[:, q * 512 : (q + 1) * 512],
                    in0=accs[q][:],
                    scalar1=colsum_s[:, 0:1],
                    scalar2=None,
                    op0=alu.add,
                )
                if q == NQ - 1:
                    nc.sync.dma_start(
                        out=yt[:, q * 512 : q * 512 + 256],
                        in_=yt_sb[:, q * 512 : q * 512 + 256],
                    )
                    nc.scalar.dma_start(
                        out=yt[:, q * 512 + 256 : (q + 1) * 512],
                        in_=yt_sb[:, q * 512 + 256 : (q + 1) * 512],
                    )
                else:
                    dma_eng = nc.sync if q % 2 == 0 else nc.scalar
                    dma_eng.dma_start(
                        out=yt[:, q * 512 : (q + 1) * 512],
                        in_=yt_sb[:, q * 512 : (q + 1) * 512],
                    )

    nc.compile()
    return nc


def _ensure_ntff_hook():
    """The image's antenv lacks axon_hooks; synthesize it so bass_utils'
    trace=True path can capture NTFF profiles via the axon .so."""
    import sys
    import types

    try:
        from antenv.axon_hooks import get_axon_ntff_profile_hook  # noqa: F401

        return
    except ImportError:
        pass

    mod = types.ModuleType("antenv.axon_hooks")
    _hook = [None]
    mod.set_axon_ntff_profile_hook = lambda h: _hook.__setitem__(0, h)
    mod.get_axon_ntff_profile_hook = lambda: _hook[0]
    sys.modules["antenv.axon_hooks"] = mod
    import antenv

    antenv.axon_hooks = mod
    try:
        from trn_agent_boot.trn_boot import _ntff_profile_via_ctypes

        mod.set_axon_ntff_profile_hook(
            _ntff_profile_via_ctypes("/opt/axon/libaxon_pjrt.so")
        )
    except Exception:
        pass

    # Keep artifact handling local — no share/S3 in this container.
    import concourse.bass_utils as bu

    bu.upload_artifacts = lambda tmpdir: tmpdir


def kernel(adjacency, features, weight, _trace=False, _tmpdir=None):
    from concourse.bass_utils import run_bass_kernel_spmd

    if _trace:
        _ensure_ntff_hook()

    if "nc" not in _CACHE:
        _CACHE["nc"] = _build_program()
    nc = _CACHE["nc"]

    adjacency = np.asarray(adjacency, dtype=np.float32)
    features = np.asarray(features, dtype=np.float32)
    weight = np.asarray(weight, dtype=np.float32)
    xsum = features.sum(axis=0, dtype=np.float64)
    x64 = features.astype(np.float64)

    in_maps = []
    for c in range(NCORES):
        # z[j, f] partition-major: [j%128, j//128, f]
        z = (x64 @ weight[c].T.astype(np.float64)).reshape(JBLK, 128, F)
        z = np.ascontiguousarray(z.transpose(1, 0, 2))          # [128, 32, F]
        z16_np = np.ascontiguousarray(z[:, :NFPK, :]).astype(np.float16)
        z8_np = (
            np.ascontiguousarray(z[:, NFPK:, :])
            .astype(np.float32)
            .astype(ml_dtypes.float8_e4m3)
        )
        # at[j, i] = 16 * (A^T - 0.5); block layout [j%128, q, j//128, i%512]
        b = (adjacency[c].T - 0.5) * ASCALE
        kt = b.reshape(JBLK, 128, NQ, 512).transpose(1, 2, 0, 3)  # [p, q, kt, i]
        a3_np = np.ascontiguousarray(kt[:, :, :NFPK, :]).reshape(
            128, NQ * NFPK * 512
        ).astype(ml_dtypes.float8_e3m4)
        a4_np = np.ascontiguousarray(kt[:, :, NFPK:, :]).reshape(
            128, NQ * NDRK * 512
        ).astype(ml_dtypes.float8_e4m3)
        cs_np = (8.0 * (weight[c].astype(np.float64) @ xsum)).astype(
            np.float32
        ).reshape(F, 1)
        in_maps.append(
            {
                "a3": a3_np,
                "a4": a4_np,
                "z16d": z16_np.reshape(128, NFPK * 128),
                "z8d": z8_np.reshape(128, NDRK * 128),
                "cs": cs_np,
            }
        )

    res = run_bass_kernel_spmd(
        nc, in_maps, core_ids=list(range(NCORES)), trace=_trace, tmpdir=_tmpdir
    )
    _CACHE["last_exec_ns"] = res.exec_time_ns
    _CACHE["last_results"] = res

    yt_sum = np.zeros((F, N), dtype=np.float32)
    for r in res.results:
        yt_sum += np.asarray(r["yt"]).astype(np.float32)
    yt_sum *= 1.0 / ASCALE
    return np.ascontiguousarray(yt_sum.T)


# revision 13
# speedup vs baseline: 1.1050x; 1.1050x over previous
"""Relational GNN layer  y = sum_r A_r @ X @ W_r^T  on 8 trn2 NeuronCores.

Sharding: relation-parallel. Core c handles relation c:
    Y_c = A_c @ Z_c,   Z_c = X @ W_c^T     (A_c: [N, N], Z_c: [N, F])
Host sums the 8 partial [N, F] outputs.

The kernel is at the per-core HBM roofline (~330 GB/s measured; ring
count does not change it), so the structure minimizes DMA bytes and
keeps the stream saturated end to end:
  - A ships as 1 byte/element fp8 after mean-centering A = 0.5 + B,
    at = fp8(16 * B); the 0.5-mean path is restored exactly via the
    host rank-1 term cs = 8 * W_c @ colsum(X).
  - Z_c is tiny (0.69 MiB vs 16.8 MiB for A) and is computed host-side
    and shipped directly as z16 (fp16) / z8 (e4m3) — no xt/wt transfer,
    no on-device Z phase gating the PE.
  - Column-block order: the full 32-k-tile contraction runs per 512-wide
    output block, so each PSUM bank finishes while A still streams and
    its copy-out + output DMA hide inside the stream (no serial tail).

Hybrid precision (per-matmul perf on trn2 PE, measured):
  - fp16 x e3m4 tiles: 512-col matmul = 518 cyc (1 k-tile).
  - e4m3 x e4m3 DoubleRow: same 518 cyc for 2 k-tiles (2 fp8
    weights/cell; full 2x, no observed penalty).
First NFPK=10 k-tiles of each contraction run accurate (fp16 x e3m4),
the remaining 22 run DoubleRow. Sim-measured end-to-end relative error:
1.847e-2 (gate 2e-2), deterministic (fixed PRNG seed). Per-block PE
time (4.5 us at 2.4 GHz, 5.4 us throttled) stays under the per-block
DMA arrival time (~5.7-6.9 us) in every observed bandwidth regime, so
early arrival jitter never turns into an unrecoverable PE backlog —
the PE keeps catch-up slack and the kernel ends with the DMA stream.

Shapes hardcoded for R=8, N=4096, F_IN=F_OUT=128.
"""

import numpy as np
import ml_dtypes

R, N, F = 8, 4096, 128
JBLK = 32                # contraction k-tiles of 128
NFPK = 10                # k-tiles per block in accurate fp16 x e3m4 mode
NDRK = JBLK - NFPK       # k-tiles per block in fp8 DoubleRow mode (pairs)
NCORES = 8
NQ = N // 512            # 8 psum banks / 512-wide output blocks
ASCALE = 16.0
NWARM = 6

_CACHE = {}


def _build_program():
    import concourse.mybir as mybir
    import concourse.tile as tile
    from concourse import bacc

    dt = mybir.dt
    alu = mybir.AluOpType
    nc = bacc.Bacc("TRN2", target_bir_lowering=False, debug=False)

    a3 = nc.dram_tensor("a3", [128, NQ * NFPK * 512], dt.float8e3, kind="ExternalInput").ap()
    a4 = nc.dram_tensor("a4", [128, NQ * NDRK * 512], dt.float8e4, kind="ExternalInput").ap()
    z16d = nc.dram_tensor("z16d", [128, NFPK * 128], dt.float16, kind="ExternalInput").ap()
    z8d = nc.dram_tensor("z8d", [128, NDRK * 128], dt.float8e4, kind="ExternalInput").ap()
    cs = nc.dram_tensor("cs", [F, 1], dt.float32, kind="ExternalInput").ap()
    yt = nc.dram_tensor("yt", [F, N], dt.float16, kind="ExternalOutput").ap()

    with tile.TileContext(nc) as tc:
        with (
            tc.sbuf_pool(name="const", bufs=1) as cpool,
            tc.sbuf_pool(name="a3blocks", bufs=6) as pa3,
            tc.sbuf_pool(name="a4blocks", bufs=6) as pa4,
            tc.psum_pool(name="yp", bufs=8) as yp,
        ):
            accs = [
                yp.tile([128, 512], dt.float32, tag="yacc", name=f"yacc{q}")
                for q in range(NQ)
            ]

            # Warm the PE p-state while the lead-in DMAs run; depends only
            # on a DVE memset.
            wdum = cpool.tile([128, 128], dt.float16)
            nc.vector.memset(wdum[:], 0.0)
            for _ in range(NWARM):
                nc.tensor.matmul(
                    accs[0][:, 0:128], lhsT=wdum[:], rhs=wdum[:],
                    start=True, stop=True,
                )

            # z tiles lead both rings (small; gate the whole PE stream).
            z16 = cpool.tile([128, NFPK, 128], dt.float16)
            nc.sync.dma_start(out=z16[:], in_=z16d)
            z8 = cpool.tile([128, NDRK, 128], dt.float8e4)
            nc.scalar.dma_start(out=z8[:], in_=z8d)
            colsum_s = cpool.tile([128, 1], dt.float32)
            nc.scalar.dma_start(out=colsum_s[:], in_=cs)

            # A block DMAs: per output block q, the fp16-mode part a3q
            # [128, NFPK, 512] rides one ring and the DoubleRow part a4q
            # [128, NDRK, 512] the other (alternating per q); the first and
            # last blocks are split finer for the stream's ramp and tail.
            # Pool bufs (6 blocks in flight) ride out bandwidth dips.
            yt_sb = cpool.tile([128, N], dt.float16)
            H3 = NFPK // 2
            H4 = NDRK // 2
            for q in range(NQ):
                a3q = pa3.tile([128, NFPK, 512], dt.float8e3, tag="a3b",
                               name=f"a3b{q}")
                a4q = pa4.tile([128, NDRK, 512], dt.float8e4, tag="a4b",
                               name=f"a4b{q}")
                e0, e1 = (nc.sync, nc.scalar) if q % 2 == 0 else (nc.scalar, nc.sync)
                o3 = q * NFPK * 512
                o4 = q * NDRK * 512
                # every block's parts split evenly across BOTH rings so the
                # rings advance in lockstep per block: the in-order PE is
                # never gated by one lagging ring while the other ring's
                # later blocks pile up in buffers. q0's a3 rides sync twice
                # (scalar starts ~1.5us later, engine preamble).
                (nc.sync if q == 0 else e0).dma_start(
                    out=a3q[:, 0:H3, :], in_=a3[:, o3 : o3 + H3 * 512])
                (nc.sync if q == 0 else e1).dma_start(
                    out=a3q[:, H3:NFPK, :],
                    in_=a3[:, o3 + H3 * 512 : o3 + NFPK * 512])
                e1.dma_start(out=a4q[:, 0:H4, :],
                             in_=a4[:, o4 : o4 + H4 * 512])
                e0.dma_start(out=a4q[:, H4:NDRK, :],
                             in_=a4[:, o4 + H4 * 512 : o4 + NDRK * 512])

                # Full contraction for output block q: acc[f, i] =
                # sum_j z[j, f] * at[j, i],  j = all 32 k-tiles.
                for kt in range(NFPK):
                    nc.tensor.matmul(
                        accs[q][:],
                        lhsT=z16[:, kt : kt + 1, :],
                        rhs=a3q[:, kt, :],
                        start=(kt == 0),
                        stop=False,
                    )
                for u in range(NDRK // 2):
                    nc.tensor.matmul(
                        accs[q][:],
                        lhsT=z8[:, 2 * u : 2 * u + 2, :],
                        rhs=a4q[:, 2 * u : 2 * u + 2, :],
                        start=False,
                        stop=(u == NDRK // 2 - 1),
                        perf_mode=mybir.MatmulPerfMode.DoubleRow,
                    )

                # copy-out block q (+cs mean correction, fp32 -> fp16) and
                # its output chunk, hidden under the next blocks' stream.
                nc.vector.tensor_scalar(
                    out=yt_sb[:, q * 512 : (q + 1) * 512],
                    in0=accs[q][:],
                    scalar1=colsum_s[:, 0:1],
                    scalar2=None,
                    op0=alu.add,
                )
                if q == NQ - 1:
                    nc.sync.dma_start(
                        out=yt[:, q * 512 : q * 512 + 256],
                        in_=yt_sb[:, q * 512 : q * 512 + 256],
                    )
                    nc.scalar.dma_start(
                        out=yt[:, q * 512 + 256 : (q + 1) * 512],
                        in_=yt_sb[:, q * 512 + 256 : (q + 1) * 512],
                    )
                else:
                    # outputs ride the gpsimd software-DGE queue: an output
                    # descriptor waiting on its copy-out must never block
                    # A-read descriptors queued behind it on the HW rings.
                    nc.gpsimd.dma_start(
                        out=yt[:, q * 512 : (q + 1) * 512],
                        in_=yt_sb[:, q * 512 : (q + 1) * 512],
                    )

    nc.compile()
    return nc


def _ensure_ntff_hook():
    """The image's antenv lacks axon_hooks; synthesize it so bass_utils'
    trace=True path can capture NTFF profiles via the axon .so."""
    import sys
    import types

    try:
        from antenv.axon_hooks import get_axon_ntff_profile_hook  # noqa: F401

        return
    except ImportError:
        pass

    mod = types.ModuleType("antenv.axon_hooks")
    _hook = [None]
    mod.set_axon_ntff_profile_hook = lambda h: _hook.__setitem__(0, h)
    mod.get_axon_ntff_profile_hook = lambda: _hook[0]
    sys.modules["antenv.axon_hooks"] = mod
    import antenv

    antenv.axon_hooks = mod
    try:
        from trn_agent_boot.trn_boot import _ntff_profile_via_ctypes

        mod.set_axon_ntff_profile_hook(
            _ntff_profile_via_ctypes("/opt/axon/libaxon_pjrt.so")
        )
    except Exception:
        pass

    # Keep artifact handling local — no share/S3 in this container.
    import concourse.bass_utils as bu

    bu.upload_artifacts = lambda tmpdir: tmpdir


def kernel(adjacency, features, weight, _trace=False, _tmpdir=None):
    from concourse.bass_utils import run_bass_kernel_spmd

    if _trace:
        _ensure_ntff_hook()

    if "nc" not in _CACHE:
        _CACHE["nc"] = _build_program()
    nc = _CACHE["nc"]

    adjacency = np.asarray(adjacency, dtype=np.float32)
    features = np.asarray(features, dtype=np.float32)
    weight = np.asarray(weight, dtype=np.float32)
    xsum = features.sum(axis=0, dtype=np.float64)
    x64 = features.astype(np.float64)

    in_maps = []
    for c in range(NCORES):
        # z[j, f] partition-major: [j%128, j//128, f]
        z = (x64 @ weight[c].T.astype(np.float64)).reshape(JBLK, 128, F)
        z = np.ascontiguousarray(z.transpose(1, 0, 2))          # [128, 32, F]
        z16_np = np.ascontiguousarray(z[:, :NFPK, :]).astype(np.float16)
        z8_np = (
            np.ascontiguousarray(z[:, NFPK:, :])
            .astype(np.float32)
            .astype(ml_dtypes.float8_e4m3)
        )
        # at[j, i] = 16 * (A^T - 0.5); block layout [j%128, q, j//128, i%512]
        b = (adjacency[c].T - 0.5) * ASCALE
        kt = b.reshape(JBLK, 128, NQ, 512).transpose(1, 2, 0, 3)  # [p, q, kt, i]
        a3_np = np.ascontiguousarray(kt[:, :, :NFPK, :]).reshape(
            128, NQ * NFPK * 512
        ).astype(ml_dtypes.float8_e3m4)
        a4_np = np.ascontiguousarray(kt[:, :, NFPK:, :]).reshape(
            128, NQ * NDRK * 512
        ).astype(ml_dtypes.float8_e4m3)
        cs_np = (8.0 * (weight[c].astype(np.float64) @ xsum)).astype(
            np.float32
        ).reshape(F, 1)
        in_maps.append(
            {
                "a3": a3_np,
                "a4": a4_np,
                "z16d": z16_np.reshape(128, NFPK * 128),
                "z8d": z8_np.reshape(128, NDRK * 128),
                "cs": cs_np,
            }
        )

    res = run_bass_kernel_spmd(
        nc, in_maps, core_ids=list(range(NCORES)), trace=_trace, tmpdir=_tmpdir
    )
    _CACHE["last_exec_ns"] = res.exec_time_ns
    _CACHE["last_results"] = res

    yt_sum = np.zeros((F, N), dtype=np.float32)
    for r in res.results:
        yt_sum += np.asarray(r["yt"]).astype(np.float32)
    yt_sum *= 1.0 / ASCALE
    return np.ascontiguousarray(yt_sum.T)
